# revision 5
# baseline (speedup 1.0000x reference)
"""Bahdanau-style attention scores kernel for Trainium2 (8 NeuronCores).

Reference computation (B=32, S=2048, ENC_H=512, DEC_H=1024):
    W_s = attn_w[:, :1024]; W_e = attn_w[:, 1024:]
    proj_s = s @ W_s.T                      # [B, 1024]
    proj_e = enc @ W_e.T                    # [B, S, 1024]
    scores = tanh(proj_s[:, None] + proj_e) @ v_w.T   # [B, S]
    out = softmax(scores, axis=1)

Strategy: data-parallel over batch (4 batches per core). Everything is
core-local, including the softmax, so there are no collectives.

On-device layout: all matmuls keep the hidden dim h on PSUM partitions:
    projT[h, s] = sum_e W_eT[e, h] * encT[e, s]
so the per-batch proj_s bias is a per-partition scalar (fused into the
ACT tanh) and the v-dot is a tiny M=1 matmul. The host passes
encoder_outputs pre-transposed to [b, E, S] (pure layout change, f32);
f32 -> bf16 conversion of the enc stream happens inside the SWDGE DMA
(cast-on-load). The small replicated weights are pre-cast to bf16 on
the host and loaded via HWDGE so the critical prologue prefix is short.
"""

import numpy as np
import ml_dtypes

import concourse.bass as bass
import concourse.tile as tile
from concourse import mybir
from concourse.bass_utils import run_bass_kernel_spmd

N_CORES = 8
B, S = 32, 2048
E = 1024  # 2*ENC_H, contraction dim of the big matmul
H = 1024  # DEC_H, hidden dim of tanh
D = 1024  # DEC_H, contraction dim of proj_s
BPC = B // N_CORES  # batches per core
P = 128
EC, HC, DC = E // P, H // P, D // P
SBLK = 512
NSB = S // SBLK

F32 = mybir.dt.float32
BF16 = mybir.dt.bfloat16
NP_BF16 = ml_dtypes.bfloat16

_cache = {}


def _split_multiwaits(nc):
    """Walrus in this toolchain rejects instructions carrying more than one
    semaphore wait ("Too many sync wait commands"). Engine queues dispatch in
    order, so moving the extra waits onto same-engine NoOps just before the
    instruction is semantically identical."""
    for fn in nc.m.functions:
        for blk in fn.blocks:
            out = []
            for inst in blk.instructions:
                si = inst.sync_info
                waits = list(si.on_wait) if si is not None and si.on_wait else []
                if len(waits) > 1:
                    for i, w in enumerate(waits[:-1]):
                        out.append(
                            mybir.InstNoOp(
                                name=f"{inst.name}-w{i}",
                                engine=inst.engine,
                                sync_info=mybir.SyncInfo(on_wait=[w], on_update=[]),
                                bass_nofuse=True,
                            )
                        )
                    si.on_wait = [waits[-1]]
                    inst.sync_info = si
                out.append(inst)
            try:
                blk.instructions = out
            except Exception:
                blk.set_instructions(out)


def _build_bass():
    nc = bass.Bass()
    enc_t = nc.dram_tensor("enc_t", [BPC, E, S], F32, kind="ExternalInput")
    w_et = nc.dram_tensor("w_et", [E, H], BF16, kind="ExternalInput")
    w_st = nc.dram_tensor("w_st", [D, H], BF16, kind="ExternalInput")
    s_t = nc.dram_tensor("s_t", [D, BPC], BF16, kind="ExternalInput")
    v_t = nc.dram_tensor("v_t", [H, 1], BF16, kind="ExternalInput")
    out = nc.dram_tensor("out", [BPC, S], F32, kind="ExternalOutput")

    Tanh = mybir.ActivationFunctionType.Tanh
    Exp = mybir.ActivationFunctionType.Exp

    with tile.TileContext(nc) as tc:
        with (
            tc.tile_pool(name="consts", bufs=1) as consts,
            tc.tile_pool(name="enc", bufs=2) as enc_pool,
            tc.tile_pool(name="tanh", bufs=4) as tanh_pool,
            tc.tile_pool(name="rows", bufs=2) as row_pool,
            tc.tile_pool(name="mmps", bufs=3, space="PSUM") as mm_psum,
            tc.tile_pool(name="scps", bufs=2, space="PSUM") as sc_psum,
            tc.tile_pool(name="psps", bufs=2, space="PSUM") as ps_psum,
        ):
            # Small bf16 weights via HWDGE (separate DGE path from the big
            # SWDGE enc stream). Issue order = need order.
            s_sb = consts.tile([P, DC, BPC], BF16)
            nc.sync.dma_start(
                out=s_sb[:], in_=s_t[:].rearrange("(dc p) b -> p dc b", p=P)
            )
            v_sb = consts.tile([P, HC, 1], BF16)
            nc.sync.dma_start(
                out=v_sb[:], in_=v_t[:].rearrange("(hc p) o -> p hc o", p=P)
            )
            ws_sb = consts.tile([P, DC, H], BF16)
            ws_view = w_st[:].rearrange("(dc p) h -> p dc h", p=P)
            for hc in range(HC):
                nc.sync.dma_start(
                    out=ws_sb[:, :, hc * P : (hc + 1) * P],
                    in_=ws_view[:, :, hc * P : (hc + 1) * P],
                )
            w_sb = consts.tile([P, EC, H], BF16)
            w_view = w_et[:].rearrange("(ec p) h -> p ec h", p=P)
            for ec in range(EC):
                nc.sync.dma_start(out=w_sb[:, ec, :], in_=w_view[:, ec, :])

            # projsT[h, b] = sum_d W_sT[d, h] * sT[d, b]  (tiny, one-time)
            projs_sb = consts.tile([P, HC, BPC], F32)
            for hc in range(HC):
                ps = ps_psum.tile([P, BPC], F32, tag="psps")
                for dc in range(DC):
                    nc.tensor.matmul(
                        ps,
                        ws_sb[:, dc, hc * P : (hc + 1) * P],
                        s_sb[:, dc, :],
                        start=(dc == 0),
                        stop=(dc == DC - 1),
                    )
                nc.vector.tensor_copy(projs_sb[:, hc, :], ps)

            for b in range(BPC):
                encT = enc_pool.tile([P, EC, S], BF16)
                # Load in s-block granularity so the first matmul group only
                # gates on the first 2 MB, not the whole 8 MB batch.
                for sb in range(NSB):
                    for ec in range(EC):
                        nc.gpsimd.dma_start(
                            out=encT[:, ec, sb * SBLK : (sb + 1) * SBLK],
                            in_=enc_t[
                                b, ec * P : (ec + 1) * P, sb * SBLK : (sb + 1) * SBLK
                            ],
                        )
                exp_row = row_pool.tile([1, S], F32, tag="exp_row")
                sums = row_pool.tile([1, NSB], F32, tag="sums")
                for sb in range(NSB):
                    sc_ps = sc_psum.tile([1, SBLK], F32, tag="scps")
                    for hc in range(HC):
                        mm_ps = mm_psum.tile([P, SBLK], F32, tag="mmps")
                        for ec in range(EC):
                            nc.tensor.matmul(
                                mm_ps,
                                w_sb[:, ec, hc * P : (hc + 1) * P],
                                encT[:, ec, sb * SBLK : (sb + 1) * SBLK],
                                start=(ec == 0),
                                stop=(ec == EC - 1),
                            )
                        th = tanh_pool.tile([P, SBLK], BF16, tag="tanh")
                        nc.scalar.activation(
                            th, mm_ps, Tanh, bias=projs_sb[:, hc, b : b + 1]
                        )
                        nc.tensor.matmul(
                            sc_ps,
                            v_sb[:, hc, :],
                            th,
                            start=(hc == 0),
                            stop=(hc == HC - 1),
                        )
                    # exp(scores) with fused per-partition running sum
                    nc.scalar.activation(
                        exp_row[:, sb * SBLK : (sb + 1) * SBLK],
                        sc_ps,
                        Exp,
                        accum_out=sums[:, sb : sb + 1],
                    )
                tot = row_pool.tile([1, 1], F32, tag="tot")
                nc.vector.reduce_sum(tot, sums, axis=mybir.AxisListType.X)
                rtot = row_pool.tile([1, 1], F32, tag="rtot")
                nc.vector.reciprocal(rtot, tot)
                out_row = row_pool.tile([1, S], F32, tag="out_row")
                nc.vector.tensor_scalar_mul(out_row, exp_row, rtot)
                nc.sync.dma_start(out=out[b : b + 1, :], in_=out_row[:])

    _split_multiwaits(nc)
    return nc


def _prep_inputs(s, encoder_outputs, attn_w, v_w):
    s = np.asarray(s, dtype=np.float32)
    enc = np.asarray(encoder_outputs, dtype=np.float32)
    attn_w = np.asarray(attn_w, dtype=np.float32)
    v_w = np.asarray(v_w, dtype=np.float32)

    w_et = np.ascontiguousarray(attn_w[:, D:].T).astype(NP_BF16)  # [E, H]
    w_st = np.ascontiguousarray(attn_w[:, :D].T).astype(NP_BF16)  # [D, H]
    v_t = np.ascontiguousarray(v_w.T).astype(NP_BF16)  # [H, 1]

    in_maps = []
    for c in range(N_CORES):
        lo, hi = c * BPC, (c + 1) * BPC
        enc_t = np.ascontiguousarray(enc[lo:hi].transpose(0, 2, 1))  # [BPC, E, S]
        s_t = np.ascontiguousarray(s[lo:hi].T).astype(NP_BF16)  # [D, BPC]
        in_maps.append(
            {"enc_t": enc_t, "w_et": w_et, "w_st": w_st, "s_t": s_t, "v_t": v_t}
        )
    return in_maps


def _run(s, encoder_outputs, attn_w, v_w, trace=False):
    if "nc" not in _cache:
        _cache["nc"] = _build_bass()
    nc = _cache["nc"]
    in_maps = _prep_inputs(s, encoder_outputs, attn_w, v_w)
    res = run_bass_kernel_spmd(nc, in_maps, list(range(N_CORES)), trace=trace)
    out = np.concatenate([res.results[c]["out"] for c in range(N_CORES)], axis=0)
    return out.astype(np.float32), res


def kernel(s, encoder_outputs, attn_w, v_w):
    out, _ = _run(s, encoder_outputs, attn_w, v_w, trace=False)
    return out


# revision 7
# speedup vs baseline: 1.1442x; 1.1442x over previous
"""Bahdanau-style attention scores kernel for Trainium2 (8 NeuronCores).

Reference computation (B=32, S=2048, ENC_H=512, DEC_H=1024):
    W_s = attn_w[:, :1024]; W_e = attn_w[:, 1024:]
    proj_s = s @ W_s.T                      # [B, 1024]
    proj_e = enc @ W_e.T                    # [B, S, 1024]
    scores = tanh(proj_s[:, None] + proj_e) @ v_w.T   # [B, S]
    out = softmax(scores, axis=1)

Strategy: data-parallel over batch (4 batches per core). Everything is
core-local, including the softmax, so there are no collectives.

On-device layout: all matmuls keep the hidden dim h on PSUM partitions:
    projT[h, s] = sum_e W_eT[e, h] * encT[e, s]
so the per-batch proj_s bias is a per-partition scalar (fused into the
ACT tanh) and the v-dot is a tiny M=1 matmul. The host passes
encoder_outputs pre-transposed to [b, E, S] (pure layout change, f32);
f32 -> bf16 conversion of the enc stream happens inside the SWDGE DMA
(cast-on-load). The small replicated weights are pre-cast to bf16 on
the host and loaded via HWDGE so the critical prologue prefix is short.
"""

import numpy as np
import ml_dtypes

import concourse.bass as bass
import concourse.tile as tile
from concourse import mybir
from concourse.bass_utils import run_bass_kernel_spmd

N_CORES = 8
B, S = 32, 2048
E = 1024  # 2*ENC_H, contraction dim of the big matmul
H = 1024  # DEC_H, hidden dim of tanh
D = 1024  # DEC_H, contraction dim of proj_s
BPC = B // N_CORES  # batches per core
P = 128
EC, HC, DC = E // P, H // P, D // P
SBLK = 512
NSB = S // SBLK

F32 = mybir.dt.float32
BF16 = mybir.dt.bfloat16
NP_BF16 = ml_dtypes.bfloat16

_cache = {}


def _split_multiwaits(nc):
    """Walrus in this toolchain rejects instructions carrying more than one
    semaphore wait ("Too many sync wait commands"). Engine queues dispatch in
    order, so moving the extra waits onto same-engine NoOps just before the
    instruction is semantically identical."""
    for fn in nc.m.functions:
        for blk in fn.blocks:
            out = []
            for inst in blk.instructions:
                si = inst.sync_info
                waits = list(si.on_wait) if si is not None and si.on_wait else []
                if len(waits) > 1:
                    for i, w in enumerate(waits[:-1]):
                        out.append(
                            mybir.InstNoOp(
                                name=f"{inst.name}-w{i}",
                                engine=inst.engine,
                                sync_info=mybir.SyncInfo(on_wait=[w], on_update=[]),
                                bass_nofuse=True,
                            )
                        )
                    si.on_wait = [waits[-1]]
                    inst.sync_info = si
                out.append(inst)
            try:
                blk.instructions = out
            except Exception:
                blk.set_instructions(out)


def _build_bass():
    nc = bass.Bass()
    enc_t = nc.dram_tensor("enc_t", [BPC, E, S], F32, kind="ExternalInput")
    w_et = nc.dram_tensor("w_et", [E, H], BF16, kind="ExternalInput")
    w_st = nc.dram_tensor("w_st", [D, H], BF16, kind="ExternalInput")
    s_t = nc.dram_tensor("s_t", [D, BPC], BF16, kind="ExternalInput")
    v_t = nc.dram_tensor("v_t", [H, 1], BF16, kind="ExternalInput")
    out = nc.dram_tensor("out", [BPC, S], F32, kind="ExternalOutput")

    Tanh = mybir.ActivationFunctionType.Tanh
    Exp = mybir.ActivationFunctionType.Exp

    with tile.TileContext(nc) as tc:
        with (
            tc.tile_pool(name="consts", bufs=1) as consts,
            tc.tile_pool(name="enc", bufs=2) as enc_pool,
            tc.tile_pool(name="tanh", bufs=4) as tanh_pool,
            tc.tile_pool(name="rows", bufs=2) as row_pool,
            tc.tile_pool(name="mmps", bufs=3, space="PSUM") as mm_psum,
            tc.tile_pool(name="scps", bufs=2, space="PSUM") as sc_psum,
            tc.tile_pool(name="psps", bufs=2, space="PSUM") as ps_psum,
        ):
            # Small bf16 weights via HWDGE, on two independent rings (SP for
            # the main weight, ACT for the proj_s inputs) so neither queues
            # behind the other, both contiguous 2KB-run APs. The big SWDGE
            # enc stream uses the third (gpsimd) path.
            w_sb = consts.tile([P, EC, H], BF16)
            w_view = w_et[:].rearrange("(ec p) h -> p ec h", p=P)
            for ec in range(EC):
                nc.sync.dma_start(out=w_sb[:, ec, :], in_=w_view[:, ec, :])
            s_sb = consts.tile([P, DC, BPC], BF16)
            nc.scalar.dma_start(
                out=s_sb[:], in_=s_t[:].rearrange("(dc p) b -> p dc b", p=P)
            )
            v_sb = consts.tile([P, HC, 1], BF16)
            nc.scalar.dma_start(
                out=v_sb[:], in_=v_t[:].rearrange("(hc p) o -> p hc o", p=P)
            )
            ws_sb = consts.tile([P, DC, H], BF16)
            nc.scalar.dma_start(
                out=ws_sb[:], in_=w_st[:].rearrange("(dc p) h -> p dc h", p=P)
            )

            # projsT[h, b] = sum_d W_sT[d, h] * sT[d, b]  (tiny, one-time)
            projs_sb = consts.tile([P, HC, BPC], F32)
            for hc in range(HC):
                ps = ps_psum.tile([P, BPC], F32, tag="psps")
                for dc in range(DC):
                    nc.tensor.matmul(
                        ps,
                        ws_sb[:, dc, hc * P : (hc + 1) * P],
                        s_sb[:, dc, :],
                        start=(dc == 0),
                        stop=(dc == DC - 1),
                    )
                nc.vector.tensor_copy(projs_sb[:, hc, :], ps)

            for b in range(BPC):
                encT = enc_pool.tile([P, EC, S], BF16)
                if b == 0:
                    # s-block granularity so the first matmul group gates on
                    # the first 2 MB, not the whole 8 MB batch.
                    for sb in range(NSB):
                        for ec in range(EC):
                            nc.gpsimd.dma_start(
                                out=encT[:, ec, sb * SBLK : (sb + 1) * SBLK],
                                in_=enc_t[
                                    b,
                                    ec * P : (ec + 1) * P,
                                    sb * SBLK : (sb + 1) * SBLK,
                                ],
                            )
                else:
                    for ec in range(EC):
                        nc.gpsimd.dma_start(
                            out=encT[:, ec, :],
                            in_=enc_t[b, ec * P : (ec + 1) * P, :],
                        )
                exp_row = row_pool.tile([1, S], F32, tag="exp_row")
                sums = row_pool.tile([1, NSB], F32, tag="sums")
                for sb in range(NSB):
                    sc_ps = sc_psum.tile([1, SBLK], F32, tag="scps")
                    for hc in range(HC):
                        mm_ps = mm_psum.tile([P, SBLK], F32, tag="mmps")
                        for ec in range(EC):
                            nc.tensor.matmul(
                                mm_ps,
                                w_sb[:, ec, hc * P : (hc + 1) * P],
                                encT[:, ec, sb * SBLK : (sb + 1) * SBLK],
                                start=(ec == 0),
                                stop=(ec == EC - 1),
                            )
                        th = tanh_pool.tile([P, SBLK], BF16, tag="tanh")
                        nc.scalar.activation(
                            th, mm_ps, Tanh, bias=projs_sb[:, hc, b : b + 1]
                        )
                        nc.tensor.matmul(
                            sc_ps,
                            v_sb[:, hc, :],
                            th,
                            start=(hc == 0),
                            stop=(hc == HC - 1),
                        )
                    # exp(scores) with fused per-partition running sum
                    nc.scalar.activation(
                        exp_row[:, sb * SBLK : (sb + 1) * SBLK],
                        sc_ps,
                        Exp,
                        accum_out=sums[:, sb : sb + 1],
                    )
                tot = row_pool.tile([1, 1], F32, tag="tot")
                nc.vector.reduce_sum(tot, sums, axis=mybir.AxisListType.X)
                rtot = row_pool.tile([1, 1], F32, tag="rtot")
                nc.vector.reciprocal(rtot, tot)
                out_row = row_pool.tile([1, S], F32, tag="out_row")
                nc.vector.tensor_scalar_mul(out_row, exp_row, rtot)
                nc.sync.dma_start(out=out[b : b + 1, :], in_=out_row[:])

    _split_multiwaits(nc)
    return nc


def _prep_inputs(s, encoder_outputs, attn_w, v_w):
    s = np.asarray(s, dtype=np.float32)
    enc = np.asarray(encoder_outputs, dtype=np.float32)
    attn_w = np.asarray(attn_w, dtype=np.float32)
    v_w = np.asarray(v_w, dtype=np.float32)

    w_et = np.ascontiguousarray(attn_w[:, D:].T).astype(NP_BF16)  # [E, H]
    w_st = np.ascontiguousarray(attn_w[:, :D].T).astype(NP_BF16)  # [D, H]
    v_t = np.ascontiguousarray(v_w.T).astype(NP_BF16)  # [H, 1]

    in_maps = []
    for c in range(N_CORES):
        lo, hi = c * BPC, (c + 1) * BPC
        enc_t = np.ascontiguousarray(enc[lo:hi].transpose(0, 2, 1))  # [BPC, E, S]
        s_t = np.ascontiguousarray(s[lo:hi].T).astype(NP_BF16)  # [D, BPC]
        in_maps.append(
            {"enc_t": enc_t, "w_et": w_et, "w_st": w_st, "s_t": s_t, "v_t": v_t}
        )
    return in_maps


def _run(s, encoder_outputs, attn_w, v_w, trace=False):
    if "nc" not in _cache:
        _cache["nc"] = _build_bass()
    nc = _cache["nc"]
    in_maps = _prep_inputs(s, encoder_outputs, attn_w, v_w)
    res = run_bass_kernel_spmd(nc, in_maps, list(range(N_CORES)), trace=trace)
    out = np.concatenate([res.results[c]["out"] for c in range(N_CORES)], axis=0)
    return out.astype(np.float32), res


def kernel(s, encoder_outputs, attn_w, v_w):
    out, _ = _run(s, encoder_outputs, attn_w, v_w, trace=False)
    return out
